# revision 1
# baseline (speedup 1.0000x reference)
"""Trainium2 Bass kernel for nn_ConsistencyConstraint (loss_fn).

Reference computation (B=4096, D=C*H*W=4096, NCLASS=10):
    ngrad_i = (g_i - min_i) / (max_i - min_i)          per-row min-max norm
    vn_i    = ngrad_i / max(||ngrad_i||, eps)
    sim     = vn @ vn.T
    xloss   = sum_{i<j, pred_i==pred_j} (1 - sim_ij) / B
    celoss  = mean cross-entropy(outputs, y)
    loss    = celoss + xloss

Key restructuring (mathematically identical; ~1e-4 rel err against the fp32
reference, which itself carries ~2e-5 fp32 noise):

1. Cosine similarity is invariant to the per-row positive scale 1/(max-min),
   so vn_i = z_i / ||z_i|| with z_i = g_i - min_i (the eps clamp is inactive:
   min-max normalized rows always have norm >= 1).
2. For same-class pairs: sum_{i<j in c} vn_i.vn_j = (||S_c||^2 - n_c) / 2
   where S_c = sum_{i in c} vn_i and sum_c n_c = B (each ||vn_i||^2 == 1), so
       xloss = (N_pairs - (sum_c ||S_c||^2 - B) / 2) / B,
       N_pairs = sum_c n_c (n_c - 1) / 2.
   This replaces the O(B^2 D) similarity matmul with an O(B D NCLASS)
   one-hot matmul.
Per-core dataflow (512 rows, 4 chunks of 128 partitions):
  - DVE:  row min reduce (two column halves to overlap the chunk DMA);
          z = g - min (fp16) on even chunks.
  - ACT:  ||z_i||^2 in ONE pass: Square activation with bias=-min_i
          (per-partition) and free-dim accumulate; z-pass on odd chunks
          (Identity with bias=-min); CE exp.
  - PE:   8 PSUM banks accumulate S = Wa^T @ Z over the 4 chunks, with
          Wa[i,c] = [argmax(outputs_i)==c] / ||z_i|| (fp16 stationary);
          PSUM is DMA'd straight to DRAM.
  - CE / argmax bookkeeping batched across chunks as single [128,4,10] ops
    using stride-0 broadcast access patterns.
Host gather: S = sum over cores, ||S_c||^2, bincount preds for N_pairs,
celoss rows = log(se) + (max_o - o_y). No device collectives.
"""

import numpy as np

import concourse.bass as bass
import concourse.mybir as mybir
import concourse.tile as tile
from concourse import bacc
from concourse.bass_utils import run_bass_kernel_spmd

N_CORES = 8
B = 4096
D = 4096  # C*H*W = 1*64*64
NCLASS = 10
ROWS_PER_CORE = B // N_CORES  # 512
P = 128  # SBUF partitions
KCH = ROWS_PER_CORE // P  # 4 row-chunks per core
NFREE = 512  # PSUM bank width (fp32)
NCH = D // NFREE  # 8 column-chunks
DH = D // 2  # DMA column-half

F32 = mybir.dt.float32
FP16 = mybir.dt.float16

# Results of the last device run (BassKernelResults) — exposed so an external
# harness can read exec_time_ns when tracing is enabled via BASS_TRACE=1.
LAST_RESULTS = None

_nc_cache = None


def _bc(ap, pattern):
    """Rebuild an AP with an explicit [step, count] pattern (for stride-0
    broadcasts along free dims)."""
    return bass.AP(tensor=ap.tensor, offset=ap.offset, ap=pattern)


def _build_bass():
    """One SPMD program, identical on all 8 cores; only the data differs."""
    nc = bacc.Bacc()

    g_in = nc.dram_tensor("g", [ROWS_PER_CORE, D], F32, kind="ExternalInput")
    o_in = nc.dram_tensor("o", [ROWS_PER_CORE, NCLASS], F32, kind="ExternalInput")
    # y as float (values 0..9), pre-laid-out [P, KCH] with [p, k] = y[k*128+p]
    y_in = nc.dram_tensor("yf", [P, KCH], F32, kind="ExternalInput")

    p_out = nc.dram_tensor("P", [NCLASS, D], F32, kind="ExternalOutput")
    se_out = nc.dram_tensor("se", [P, KCH], F32, kind="ExternalOutput")
    dm_out = nc.dram_tensor("dm", [P, KCH], F32, kind="ExternalOutput")
    pred_out = nc.dram_tensor("pred", [P, KCH], F32, kind="ExternalOutput")

    iota_const = nc.inline_tensor(
        np.tile(np.arange(NCLASS, dtype=np.float32), (P, 1)), name="iota10"
    )

    with tile.TileContext(nc) as tc:
        with (
            tc.tile_pool(name="gpool", bufs=4) as gpool,
            tc.tile_pool(name="zpool", bufs=4) as zpool,
            tc.tile_pool(name="jpool", bufs=2) as jpool,
            tc.tile_pool(name="small", bufs=4) as small,
            tc.tile_pool(name="singles", bufs=1) as singles,
            tc.tile_pool(name="outp", bufs=1) as outp,
            tc.tile_pool(name="psum", bufs=1, space="PSUM") as psum,
        ):
            # g chunk loads first — they own the DMA pipes from t=0.
            gts = []
            for k in range(KCH):
                gt = gpool.tile([P, D], F32, tag="gt", name=f"gt{k}")
                rows = slice(k * P, (k + 1) * P)
                nc.sync.dma_start(out=gt[:, :DH], in_=g_in[rows, :DH])
                nc.sync.dma_start(out=gt[:, DH:], in_=g_in[rows, DH:])
                gts.append(gt)

            iota_sb = singles.tile([P, NCLASS], F32)
            nc.sync.dma_start(out=iota_sb, in_=iota_const[:, :])
            yf_sb = singles.tile([P, KCH], F32)
            nc.sync.dma_start(out=yf_sb, in_=y_in[:, :])
            # o as [p, k, c] = outputs[k*128+p, c]
            o_all = singles.tile([P, KCH, NCLASS], F32)
            nc.sync.dma_start(
                out=o_all, in_=o_in.rearrange("(k p) c -> p k c", p=P)
            )

            se_sb = outp.tile([P, KCH], F32)
            dm_sb = outp.tile([P, KCH], F32)
            pred_sb = outp.tile([P, KCH], F32)
            p_sb = outp.tile([NCLASS, D], F32)

            acc = [
                psum.tile([NCLASS, NFREE], F32, tag=f"acc{n}", name=f"acc{n}")
                for n in range(NCH)
            ]

            # ---- batched argmax one-hot + CE bookkeeping (all 4 chunks) ----
            mo_all = small.tile([P, KCH], F32)
            nc.vector.tensor_reduce(
                mo_all, o_all, axis=mybir.AxisListType.X, op=mybir.AluOpType.max
            )
            mo_b = _bc(mo_all[:, :], [*mo_all[:, :].ap, [0, NCLASS]])
            eq_all = small.tile([P, KCH, NCLASS], FP16)
            nc.vector.tensor_tensor(
                eq_all, o_all, mo_b, op=mybir.AluOpType.is_equal
            )

            iota_b = _bc(
                iota_sb[:, :],
                [iota_sb[:, :].ap[0], [0, KCH], iota_sb[:, :].ap[1]],
            )
            # pred_i = sum_c c * onehot[i,c]  (ties have prob ~0 for randn)
            pp_all = small.tile([P, KCH, NCLASS], F32)
            nc.vector.tensor_tensor(pp_all, eq_all, iota_b, op=mybir.AluOpType.mult)
            nc.vector.tensor_reduce(
                pred_sb, pp_all, axis=mybir.AxisListType.X, op=mybir.AluOpType.add
            )

            # CE: se = sum_c exp(o - max_o); dm = max_o - o[y]
            emo = small.tile([P, KCH, NCLASS], F32)
            nc.vector.tensor_tensor(emo, o_all, mo_b, op=mybir.AluOpType.subtract)
            et = small.tile([P, KCH, NCLASS], F32)
            nc.scalar.activation(et, emo, mybir.ActivationFunctionType.Exp)
            nc.vector.tensor_reduce(
                se_sb, et, axis=mybir.AxisListType.X, op=mybir.AluOpType.add
            )
            yf_b = _bc(yf_sb[:, :], [*yf_sb[:, :].ap, [0, NCLASS]])
            ohy = small.tile([P, KCH, NCLASS], F32)
            nc.vector.tensor_tensor(ohy, iota_b, yf_b, op=mybir.AluOpType.is_equal)
            oyp = small.tile([P, KCH, NCLASS], F32)
            nc.vector.tensor_tensor(oyp, o_all, ohy, op=mybir.AluOpType.mult)
            oy_all = small.tile([P, KCH], F32)
            nc.vector.tensor_reduce(
                oy_all, oyp, axis=mybir.AxisListType.X, op=mybir.AluOpType.add
            )
            nc.vector.tensor_sub(dm_sb, mo_all, oy_all)

            # ---- main per-chunk pipeline over grad ----
            for k in range(KCH):
                gt = gts[k]

                # per-half min (each waits only its half's DMA), then combine
                mnh = small.tile([P, 2], F32, tag="mnh")
                nc.vector.tensor_reduce(
                    mnh[:, 0:1], gt[:, :DH], axis=mybir.AxisListType.X,
                    op=mybir.AluOpType.min,
                )
                nc.vector.tensor_reduce(
                    mnh[:, 1:2], gt[:, DH:], axis=mybir.AxisListType.X,
                    op=mybir.AluOpType.min,
                )
                mn = small.tile([P, 1], F32, tag="mn")
                nc.vector.tensor_tensor(
                    mn, mnh[:, 0:1], mnh[:, 1:2], op=mybir.AluOpType.min
                )
                negm = small.tile([P, 1], F32, tag="negm")
                nc.vector.tensor_scalar_mul(negm, mn, -1.0)

                # z = g - min (fp16) on DVE (keeps ACT's activation-table
                # stable: Square/Sqrt only, no Identity swaps)
                zt = zpool.tile([P, D], FP16, tag="zt")
                nc.vector.tensor_scalar(
                    zt, gt, scalar1=mn, scalar2=None,
                    op0=mybir.AluOpType.subtract,
                )

                # ||z||^2 = sum((g - min)^2) in one ACT pass
                junk = jpool.tile([P, D], FP16, tag="junk")
                ssq = small.tile([P, 1], F32, tag="ssq")
                nc.scalar.activation(
                    junk,
                    gt,
                    mybir.ActivationFunctionType.Square,
                    bias=negm,
                    accum_out=ssq,
                )
                u = small.tile([P, 1], F32, tag="u")
                nc.scalar.activation(u, ssq, mybir.ActivationFunctionType.Sqrt)
                rs = small.tile([P, 1], F32, tag="rs")
                nc.vector.reciprocal(rs, u)

                # wa = onehot * (1/||z||), fp16 stationary operand
                wa = small.tile([P, NCLASS], FP16, tag="wa")
                nc.vector.tensor_scalar_mul(wa, eq_all[:, k, :], rs)

                for n in range(NCH):
                    nc.tensor.matmul(
                        acc[n][:, :],
                        wa,
                        zt[:, n * NFREE : (n + 1) * NFREE],
                        start=(k == 0),
                        stop=(k == KCH - 1),
                    )

            # ---- drain PSUM -> SBUF -> DRAM (copies split across engines) ----
            for n in range(NCH):
                dst = p_sb[:, n * NFREE : (n + 1) * NFREE]
                if n % 2 == 0:
                    nc.scalar.copy(dst, acc[n])
                else:
                    nc.vector.tensor_copy(dst, acc[n])
            nc.sync.dma_start(out=p_out[:, :], in_=p_sb)
            nc.sync.dma_start(out=se_out[:, :], in_=se_sb)
            nc.sync.dma_start(out=dm_out[:, :], in_=dm_sb)
            nc.sync.dma_start(out=pred_out[:, :], in_=pred_sb)

    nc.compile()
    return nc


def kernel(**inputs) -> np.ndarray:
    global LAST_RESULTS, _nc_cache

    outputs = np.ascontiguousarray(np.asarray(inputs["outputs"], dtype=np.float32))
    grad = np.asarray(inputs["grad"], dtype=np.float32).reshape(B, D)
    y = np.asarray(inputs["y"])

    if _nc_cache is None:
        _nc_cache = _build_bass()
    nc = _nc_cache

    yf = y.astype(np.float32)
    in_maps = []
    for c in range(N_CORES):
        sl = slice(c * ROWS_PER_CORE, (c + 1) * ROWS_PER_CORE)
        in_maps.append(
            {
                "g": np.ascontiguousarray(grad[sl]),
                "o": np.ascontiguousarray(outputs[sl]),
                # [p, k] = y[row k*128+p] to match the per-chunk partition layout
                "yf": np.ascontiguousarray(yf[sl].reshape(KCH, P).T),
            }
        )

    res = run_bass_kernel_spmd(nc, in_maps, core_ids=list(range(N_CORES)))
    LAST_RESULTS = res
    results = res.results

    # ---- host gather / unshard ----
    s_full = np.zeros((NCLASS, D), dtype=np.float64)
    ce_sum = 0.0
    preds = []
    for r in results:
        s_full += r["P"].astype(np.float64)
        ce_sum += float((np.log(r["se"].astype(np.float64)) + r["dm"]).sum())
        preds.append(r["pred"].astype(np.int64).reshape(-1))
    pred = np.concatenate(preds)
    counts = np.bincount(pred, minlength=max(NCLASS, int(pred.max()) + 1))

    n_pairs = float(
        (counts.astype(np.float64) * (counts.astype(np.float64) - 1) / 2).sum()
    )
    xsum = float((s_full * s_full).sum())
    xloss = (n_pairs - (xsum - B) / 2.0) / B
    celoss = ce_sum / B
    return np.float32(celoss + xloss)



# revision 2
# speedup vs baseline: 1.0959x; 1.0959x over previous
"""Trainium2 Bass kernel for nn_ConsistencyConstraint (loss_fn).

Reference computation (B=4096, D=C*H*W=4096, NCLASS=10):
    ngrad_i = (g_i - min_i) / (max_i - min_i)          per-row min-max norm
    vn_i    = ngrad_i / max(||ngrad_i||, eps)
    sim     = vn @ vn.T
    xloss   = sum_{i<j, pred_i==pred_j} (1 - sim_ij) / B
    celoss  = mean cross-entropy(outputs, y)
    loss    = celoss + xloss

Restructuring (mathematically identical; ~1e-4 rel err vs the fp32 reference):

1. Cosine similarity is invariant to the per-row positive scale 1/(max-min),
   so vn_i = z_i / ||z_i|| with z_i = g_i - min_i (eps clamp inactive).
2. For same-class pairs: sum_{i<j in c} vn_i.vn_j = (||S_c||^2 - n_c) / 2 with
   S_c = sum_{i in c} vn_i, so
       xloss = (N_pairs - (sum_c ||S_c||^2 - B) / 2) / B.
   This replaces the O(B^2 D) similarity matmul with an O(B D NCLASS)
   one-hot matmul.
3. The min subtraction commutes with the matmul:
       S_c = sum_i wa_ic g_i  -  (sum_i wa_ic min_i) * ones(D),
   so the PE streams RAW g (as float32r, full PE rate, ~tf32 precision —
   no fp16 conversion pass needed) and the rank-1 min term is applied on
   the host from the (tiny) shipped min / wa tensors.

Per-core dataflow (512 rows = 4 chunks of 128 partitions, g streamed in
column halves):
  - DVE:  row-min per half (overlaps the DMA stream), combine; reciprocal;
          wa = onehot * (1/||z||) rounded to f32r.
  - ACT:  ssq = ||z||^2 in ONE pass: Square(g, bias=-min) with free-dim
          accumulate (junk main out goes to a broadcast dummy); sqrt.
  - PE:   8 PSUM banks accumulate S' = Wa^T @ G (f32r) over the 4 chunks.
  - argmax/onehot, cross-entropy, bincount and the final assembly are
    O(B*NCLASS) host glue.
"""

import numpy as np

import concourse.bass as bass
import concourse.mybir as mybir
import concourse.tile as tile
from concourse import bacc
from concourse.bass_utils import run_bass_kernel_spmd

N_CORES = 8
B = 4096
D = 4096  # C*H*W = 1*64*64
NCLASS = 10
ROWS_PER_CORE = B // N_CORES  # 512
P = 128  # SBUF partitions
KCH = ROWS_PER_CORE // P  # 4 row-chunks per core
NFREE = 512  # PSUM bank width (fp32)
NCH = D // NFREE  # 8 column-chunks
DH = D // 2  # DMA column-half

F32 = mybir.dt.float32
F32R = mybir.dt.float32r
FP16 = mybir.dt.float16

# Results of the last device run (BassKernelResults) — exposed so an external
# harness can read exec_time_ns when tracing is enabled via BASS_TRACE=1.
LAST_RESULTS = None

_nc_cache = None


def _build_bass():
    """One SPMD program, identical on all 8 cores; only the data differs."""
    nc = bacc.Bacc()

    g_in = nc.dram_tensor("g", [ROWS_PER_CORE, D], F32R, kind="ExternalInput")
    oh_in = nc.dram_tensor("oh", [P, KCH * NCLASS], F32, kind="ExternalInput")

    s_out = nc.dram_tensor("S", [NCLASS, D], F32, kind="ExternalOutput")
    mn_out = nc.dram_tensor("mn", [P, KCH], F32, kind="ExternalOutput")
    wa_out = nc.dram_tensor("wa", [P, KCH * NCLASS], F32, kind="ExternalOutput")

    with tile.TileContext(nc) as tc:
        with (
            tc.tile_pool(name="gpool", bufs=4) as gpool,
            tc.tile_pool(name="small", bufs=4) as small,
            tc.tile_pool(name="singles", bufs=1) as singles,
            tc.tile_pool(name="outp", bufs=1) as outp,
            tc.tile_pool(name="psum", bufs=1, space="PSUM") as psum,
        ):
            # g chunk loads first — they own the DMA pipes from t=0.
            gts = []
            for k in range(KCH):
                gt = gpool.tile([P, D], F32R, tag="gt", name=f"gt{k}")
                rows = slice(k * P, (k + 1) * P)
                nc.sync.dma_start(out=gt[:, :DH], in_=g_in[rows, :DH])
                nc.sync.dma_start(out=gt[:, DH:], in_=g_in[rows, DH:])
                gts.append(gt)

            oh_sb = singles.tile([P, KCH * NCLASS], F32)
            nc.sync.dma_start(out=oh_sb, in_=oh_in[:, :])

            # ACT table warmups (Square / Sqrt / Copy each cost a ~1.3us
            # table load at first use — pay them during the DMA stream).
            wsq = singles.tile([P, 1], FP16)
            nc.scalar.activation(wsq, oh_sb[:, 0:1], mybir.ActivationFunctionType.Square)
            wsr = singles.tile([P, 1], F32)
            nc.scalar.activation(wsr, oh_sb[:, 0:1], mybir.ActivationFunctionType.Sqrt)
            wcp = singles.tile([P, 1], F32)
            nc.scalar.copy(wcp, oh_sb[:, 0:1])

            s_sb = outp.tile([NCLASS, D], F32)
            mn_sb = outp.tile([P, KCH], F32)
            wa_sb = outp.tile([P, KCH * NCLASS], F32R)
            junk = outp.tile([P, 1], FP16)  # broadcast sink for ACT main out

            acc = [
                psum.tile([NCLASS, NFREE], F32, tag=f"acc{n}", name=f"acc{n}")
                for n in range(NCH)
            ]

            for k in range(KCH):
                gt = gts[k]
                gf = gt.bitcast(F32)

                # per-half min (each waits only its half's DMA), then combine
                mnh = small.tile([P, 2], F32, tag="mnh")
                nc.vector.tensor_reduce(
                    mnh[:, 0:1], gf[:, :DH], axis=mybir.AxisListType.X,
                    op=mybir.AluOpType.min,
                )
                nc.vector.tensor_reduce(
                    mnh[:, 1:2], gf[:, DH:], axis=mybir.AxisListType.X,
                    op=mybir.AluOpType.min,
                )
                mn = mn_sb[:, k : k + 1]
                nc.vector.tensor_tensor(
                    mn, mnh[:, 0:1], mnh[:, 1:2], op=mybir.AluOpType.min
                )
                negm = small.tile([P, 1], F32, tag="negm")
                nc.vector.tensor_scalar_mul(negm, mn, -1.0)

                # ssq = ||g - min||^2 in one ACT pass (fp32 exact, junk out)
                ssq = small.tile([P, 1], F32, tag="ssq")
                nc.scalar.activation(
                    junk.broadcast_to(gt.shape),
                    gf,
                    mybir.ActivationFunctionType.Square,
                    bias=negm,
                    accum_out=ssq,
                )
                u = small.tile([P, 1], F32, tag="u")
                nc.scalar.activation(u, ssq, mybir.ActivationFunctionType.Sqrt)
                rs = small.tile([P, 1], F32, tag="rs")
                nc.vector.reciprocal(rs, u)

                # wa = onehot * (1/||z||), rounded to f32r for the PE
                wa = wa_sb[:, k * NCLASS : (k + 1) * NCLASS]
                nc.vector.tensor_scalar_mul(
                    wa, oh_sb[:, k * NCLASS : (k + 1) * NCLASS], rs
                )

                for n in range(NCH):
                    nc.tensor.matmul(
                        acc[n][:, :],
                        wa,
                        gt[:, n * NFREE : (n + 1) * NFREE],
                        start=(k == 0),
                        stop=(k == KCH - 1),
                    )

            # ---- drain PSUM -> SBUF -> DRAM (copies split across engines) ----
            for n in range(NCH):
                dst = s_sb[:, n * NFREE : (n + 1) * NFREE]
                if n % 2 == 0:
                    nc.vector.tensor_copy(dst, acc[n])
                else:
                    nc.scalar.copy(dst, acc[n])
                if n == NCH // 2 - 1:
                    nc.sync.dma_start(
                        out=s_out[:, : D // 2], in_=s_sb[:, : D // 2]
                    )
            nc.sync.dma_start(out=s_out[:, D // 2 :], in_=s_sb[:, D // 2 :])
            nc.sync.dma_start(out=mn_out[:, :], in_=mn_sb)
            nc.sync.dma_start(out=wa_out[:, :], in_=wa_sb.bitcast(F32))

    nc.compile()
    return nc


def kernel(**inputs) -> np.ndarray:
    global LAST_RESULTS, _nc_cache

    outputs = np.asarray(inputs["outputs"], dtype=np.float32)
    grad = np.asarray(inputs["grad"], dtype=np.float32).reshape(B, D)
    y = np.asarray(inputs["y"]).astype(np.int64)

    if _nc_cache is None:
        _nc_cache = _build_bass()
    nc = _nc_cache

    # host: predicted class -> one-hot (O(B*NCLASS), tiny)
    pred = np.argmax(outputs, axis=1)
    oh_full = (pred[:, None] == np.arange(NCLASS)[None, :]).astype(np.float32)

    in_maps = []
    for c in range(N_CORES):
        sl = slice(c * ROWS_PER_CORE, (c + 1) * ROWS_PER_CORE)
        # oh laid out [p, k*NCLASS+c] to match the per-chunk partition layout
        oh_core = (
            oh_full[sl]
            .reshape(KCH, P, NCLASS)
            .transpose(1, 0, 2)
            .reshape(P, KCH * NCLASS)
        )
        in_maps.append(
            {
                "g": np.ascontiguousarray(grad[sl]),
                "oh": np.ascontiguousarray(oh_core),
            }
        )

    res = run_bass_kernel_spmd(nc, in_maps, core_ids=list(range(N_CORES)))
    LAST_RESULTS = res
    results = res.results

    # ---- host gather / unshard ----
    s_full = np.zeros((NCLASS, D), dtype=np.float64)
    m_c = np.zeros(NCLASS, dtype=np.float64)
    for r in results:
        s_full += r["S"].astype(np.float64)
        mn = r["mn"].astype(np.float64)  # [P, KCH]
        wa = r["wa"].astype(np.float64).reshape(P, KCH, NCLASS)
        # rank-1 min correction: m_c += sum_{p,k} wa[p,k,c] * mn[p,k]
        m_c += np.einsum("pkc,pk->c", wa, mn)
    s_full -= m_c[:, None]

    counts = np.bincount(pred, minlength=NCLASS).astype(np.float64)
    n_pairs = float((counts * (counts - 1) / 2).sum())
    xsum = float((s_full * s_full).sum())
    xloss = (n_pairs - (xsum - B) / 2.0) / B

    o64 = outputs.astype(np.float64)
    mo = o64.max(axis=1)
    se = np.exp(o64 - mo[:, None]).sum(axis=1)
    celoss = float((np.log(se) + mo - o64[np.arange(B), y]).sum()) / B

    return np.float32(celoss + xloss)


# revision 4
# speedup vs baseline: 1.1589x; 1.0575x over previous
"""Trainium2 Bass kernel for nn_ConsistencyConstraint (loss_fn).

Reference computation (B=4096, D=C*H*W=4096, NCLASS=10):
    ngrad_i = (g_i - min_i) / (max_i - min_i)          per-row min-max norm
    vn_i    = ngrad_i / max(||ngrad_i||, eps)
    sim     = vn @ vn.T
    xloss   = sum_{i<j, pred_i==pred_j} (1 - sim_ij) / B
    celoss  = mean cross-entropy(outputs, y)
    loss    = celoss + xloss

Restructuring (mathematically identical; ~1e-4 rel err vs the fp32 reference):

1. Cosine similarity is invariant to the per-row positive scale 1/(max-min),
   so vn_i = z_i / ||z_i|| with z_i = g_i - min_i (eps clamp inactive).
2. For same-class pairs: sum_{i<j in c} vn_i.vn_j = (||S_c||^2 - n_c) / 2 with
   S_c = sum_{i in c} vn_i, so
       xloss = (N_pairs - (sum_c ||S_c||^2 - B) / 2) / B.
   This replaces the O(B^2 D) similarity matmul with an O(B D NCLASS)
   one-hot matmul.
3. The min subtraction commutes with the matmul:
       S_c = sum_i wa_ic g_i  -  (sum_i wa_ic min_i) * ones(D),
   so the PE streams RAW g (as float32r, full PE rate, ~tf32 precision —
   no fp16 conversion pass needed) and the rank-1 min term is applied on
   the host from the (tiny) shipped min / wa tensors.

Per-core dataflow (512 rows = 4 chunks of 128 partitions; g streamed in
column pieces sized so the DVE min-reduce rate matches the DMA stream rate,
with a small first piece for an early start and a small last piece for a
short tail):
  - DVE:  row-min per piece (overlaps the DMA stream), combine; reciprocal;
          wa = onehot * (1/||z||) rounded to f32r.
  - ACT:  ssq = ||z||^2 in ONE pass: Square(g, bias=-min) with free-dim
          accumulate (junk main out goes to a broadcast dummy); sqrt.
          Activation tables are warmed at t~0 off a memset tile.
  - PE:   8 PSUM banks accumulate S' = Wa^T @ G (f32r) over the 4 chunks.
  - argmax/onehot, cross-entropy, bincount and the final assembly are
    O(B*NCLASS) host glue.
"""

import numpy as np

import concourse.bass as bass
import concourse.mybir as mybir
import concourse.tile as tile
from concourse import bacc
from concourse.bass_utils import run_bass_kernel_spmd

N_CORES = 8
B = 4096
D = 4096  # C*H*W = 1*64*64
NCLASS = 10
ROWS_PER_CORE = B // N_CORES  # 512
P = 128  # SBUF partitions
KCH = ROWS_PER_CORE // P  # 4 row-chunks per core
NFREE = 512  # PSUM bank width (fp32)
NCH = D // NFREE  # 8 column-chunks

F32 = mybir.dt.float32
F32R = mybir.dt.float32r
FP16 = mybir.dt.float16

# column pieces per chunk: small first piece (early DVE start), small last
# piece on the final chunk (short min tail)
PIECES = [
    [512, 1536, 2048],
    [2048, 2048],
    [2048, 2048],
    [2048, 1536, 512],
]

# Results of the last device run (BassKernelResults) — exposed so an external
# harness can read exec_time_ns when tracing is enabled via BASS_TRACE=1.
LAST_RESULTS = None

_nc_cache = None


def _build_bass():
    """One SPMD program, identical on all 8 cores; only the data differs."""
    nc = bacc.Bacc()

    g_in = nc.dram_tensor("g", [ROWS_PER_CORE, D], F32R, kind="ExternalInput")
    oh_in = nc.dram_tensor("oh", [P, KCH * NCLASS], F32, kind="ExternalInput")

    s_out = nc.dram_tensor("S", [NCLASS, D], F32, kind="ExternalOutput")
    mn_out = nc.dram_tensor("mn", [P, KCH + 1], F32, kind="ExternalOutput")
    wa_out = nc.dram_tensor("wa", [P, KCH * NCLASS], F32, kind="ExternalOutput")

    with tile.TileContext(nc) as tc:
        with (
            tc.tile_pool(name="gpool", bufs=4) as gpool,
            tc.tile_pool(name="small", bufs=4) as small,
            tc.tile_pool(name="singles", bufs=1) as singles,
            tc.tile_pool(name="outp", bufs=1) as outp,
            tc.tile_pool(name="psum", bufs=1, space="PSUM") as psum,
        ):
            # ACT table warmups (Square / Sqrt / Copy each cost a ~1.3us
            # table load at first use — pay them at t~0 off a memset tile,
            # before any data dependencies exist).
            with tc.high_priority():
                wsrc = singles.tile([P, 1], F32)
                nc.gpsimd.memset(wsrc, 1.0)
                wsq = singles.tile([P, 1], FP16)
                nc.scalar.activation(
                    wsq, wsrc, mybir.ActivationFunctionType.Square
                )
                wsr = singles.tile([P, 1], F32)
                nc.scalar.activation(
                    wsr, wsrc, mybir.ActivationFunctionType.Sqrt
                )
                wcp = singles.tile([P, 1], F32)
                nc.scalar.copy(wcp, wsrc)

            # g piece loads first — they own the DMA pipes from t=0.
            gts = []
            for k in range(KCH):
                gt = gpool.tile([P, D], F32R, tag="gt", name=f"gt{k}")
                rows = slice(k * P, (k + 1) * P)
                col = 0
                for w in PIECES[k]:
                    nc.sync.dma_start(
                        out=gt[:, col : col + w], in_=g_in[rows, col : col + w]
                    )
                    col += w
                gts.append(gt)

            oh_sb = singles.tile([P, KCH * NCLASS], F32)
            nc.sync.dma_start(out=oh_sb, in_=oh_in[:, :])

            s_sb = outp.tile([NCLASS, D], F32)
            mn_sb = outp.tile([P, KCH + 1], F32)
            wa_sb = outp.tile([P, KCH * NCLASS], F32R)
            junk = outp.tile([P, 1], FP16)  # broadcast sink for ACT main out

            acc = [
                psum.tile([NCLASS, NFREE], F32, tag=f"acc{n}", name=f"acc{n}")
                for n in range(NCH)
            ]

            for k in range(KCH):
                gt = gts[k]
                gf = gt.bitcast(F32)
                np_k = len(PIECES[k])

                # per-piece min (each waits only its piece's DMA)
                mnh = small.tile([P, np_k], F32, tag="mnh", name=f"mnh{k}")
                col = 0
                for j, w in enumerate(PIECES[k]):
                    nc.vector.tensor_reduce(
                        mnh[:, j : j + 1],
                        gf[:, col : col + w],
                        axis=mybir.AxisListType.X,
                        op=mybir.AluOpType.min,
                    )
                    col += w

                # everything downstream of the reduces is scheduled at high
                # priority so the tile scheduler does not defer it behind
                # later chunks' bulk reduces (which starves the PE).
                with tc.high_priority():
                    mn = mn_sb[:, k : k + 1]
                    nc.vector.tensor_reduce(
                        mn, mnh, axis=mybir.AxisListType.X, op=mybir.AluOpType.min
                    )
                    negm = small.tile([P, 1], F32, tag="negm")
                    nc.vector.tensor_scalar_mul(negm, mn, -1.0)

                    # ssq = ||g - min||^2 in one ACT pass (fp32, junk out)
                    ssq = small.tile([P, 1], F32, tag="ssq")
                    nc.scalar.activation(
                        junk.broadcast_to(gt.shape),
                        gf,
                        mybir.ActivationFunctionType.Square,
                        bias=negm,
                        accum_out=ssq,
                    )
                    u = small.tile([P, 1], F32, tag="u")
                    nc.scalar.activation(
                        u, ssq, mybir.ActivationFunctionType.Sqrt
                    )
                    rs = small.tile([P, 1], F32, tag="rs")
                    nc.vector.reciprocal(rs, u)
                    if k == 0:
                        # keep the spare mn column defined
                        nc.vector.tensor_copy(mn_sb[:, KCH : KCH + 1], ssq)

                    # wa = onehot * (1/||z||), rounded to f32r for the PE
                    wa = wa_sb[:, k * NCLASS : (k + 1) * NCLASS]
                    nc.vector.tensor_scalar_mul(
                        wa, oh_sb[:, k * NCLASS : (k + 1) * NCLASS], rs
                    )

                    for n in range(NCH):
                        nc.tensor.matmul(
                            acc[n][:, :],
                            wa,
                            gt[:, n * NFREE : (n + 1) * NFREE],
                            start=(k == 0),
                            stop=(k == KCH - 1),
                        )

            # ---- drain PSUM -> SBUF -> DRAM (copies split across engines) ----
            with tc.high_priority():
                for n in range(NCH):
                    dst = s_sb[:, n * NFREE : (n + 1) * NFREE]
                    if n % 2 == 0:
                        nc.vector.tensor_copy(dst, acc[n])
                    else:
                        nc.scalar.copy(dst, acc[n])
                    if n == NCH // 2 - 1:
                        nc.sync.dma_start(
                            out=s_out[:, : D // 2], in_=s_sb[:, : D // 2]
                        )
                nc.sync.dma_start(out=s_out[:, D // 2 :], in_=s_sb[:, D // 2 :])
                nc.sync.dma_start(out=mn_out[:, :], in_=mn_sb)
                nc.sync.dma_start(out=wa_out[:, :], in_=wa_sb.bitcast(F32))

    nc.compile()
    return nc


def kernel(**inputs) -> np.ndarray:
    global LAST_RESULTS, _nc_cache

    outputs = np.asarray(inputs["outputs"], dtype=np.float32)
    grad = np.asarray(inputs["grad"], dtype=np.float32).reshape(B, D)
    y = np.asarray(inputs["y"]).astype(np.int64)

    if _nc_cache is None:
        _nc_cache = _build_bass()
    nc = _nc_cache

    # host: predicted class -> one-hot (O(B*NCLASS), tiny)
    pred = np.argmax(outputs, axis=1)
    oh_full = (pred[:, None] == np.arange(NCLASS)[None, :]).astype(np.float32)

    in_maps = []
    for c in range(N_CORES):
        sl = slice(c * ROWS_PER_CORE, (c + 1) * ROWS_PER_CORE)
        # oh laid out [p, k*NCLASS+c] to match the per-chunk partition layout
        oh_core = (
            oh_full[sl]
            .reshape(KCH, P, NCLASS)
            .transpose(1, 0, 2)
            .reshape(P, KCH * NCLASS)
        )
        in_maps.append(
            {
                "g": np.ascontiguousarray(grad[sl]),
                "oh": np.ascontiguousarray(oh_core),
            }
        )

    res = run_bass_kernel_spmd(nc, in_maps, core_ids=list(range(N_CORES)))
    LAST_RESULTS = res
    results = res.results

    # ---- host gather / unshard ----
    s_full = np.zeros((NCLASS, D), dtype=np.float64)
    m_c = np.zeros(NCLASS, dtype=np.float64)
    for r in results:
        s_full += r["S"].astype(np.float64)
        mn = r["mn"][:, :KCH].astype(np.float64)  # [P, KCH]
        wa = r["wa"].astype(np.float64).reshape(P, KCH, NCLASS)
        # rank-1 min correction: m_c += sum_{p,k} wa[p,k,c] * mn[p,k]
        m_c += np.einsum("pkc,pk->c", wa, mn)
    s_full -= m_c[:, None]

    counts = np.bincount(pred, minlength=NCLASS).astype(np.float64)
    n_pairs = float((counts * (counts - 1) / 2).sum())
    xsum = float((s_full * s_full).sum())
    xloss = (n_pairs - (xsum - B) / 2.0) / B

    o64 = outputs.astype(np.float64)
    mo = o64.max(axis=1)
    se = np.exp(o64 - mo[:, None]).sum(axis=1)
    celoss = float((np.log(se) + mo - o64[np.arange(B), y]).sum()) / B

    return np.float32(celoss + xloss)
